# revision 25
# baseline (speedup 1.0000x reference)
"""GCEncoder (RGCN basis-decomposition conv + mean aggregation + Dense/BN/ReLU)
as a Bass/Tile kernel on 8 Trainium2 NeuronCores.

Math (reference):
  W[r]  = sum_b comp[r,b] * basis[b]                    [R, N, H0]
  h[r]  = x @ W[r]                                      [R, N, H0]
  agg[d] = sum_r (1/cnt[d,r]) * sum_{e: dst=d, type=r} h[r, src_e]
  feats = agg + x @ root + bias
  z     = feats @ fc_w.T ; per-row batchnorm over H1 + gamma/beta + relu
  out   = (z[:U], z[U:]) stacked -> [2, U, H1]

Everything before the BN is linear in the H0 axis, so fc_w is folded into
the weights on the host: W'[r] = W[r] @ fc_w.T (4096 x 75), root' =
root @ fc_w.T, bias' = bias @ fc_w.T.  The device only moves 75-wide
features (~6.7x fewer matmul FLOPs than the unfolded form):

  z[d] = sum_r (1/cnt[d,r]) * (Mcnt_r[d,:] @ h'_r) + x[d] @ root' + bias'

with Mcnt_r the integer edge-multiplicity matrix (exact in fp8e4m3, half
the HBM bytes of a bf16 weighted adjacency) and h'_r = x @ W'_r.

Scheduling facts learned from perfetto traces on this stack:
  - the collective trigger waits for ALL in-flight hardware-DGE DMA to
    drain, so bulk at4 streaming rides the software-DGE (gpsimd) queue,
    which is exempt; the tiny h_cr stores ride the hardware queue so
    they drain fast and never queue behind at4 (the single-AG baseline
    lost ~35us to exactly that);
  - a collective costs ~11us trigger->mesh plus wire time; TWO half
    AllGathers pipeline: AG1 (m-tiles 0,1) triggers mid-phase-A and its
    wire time hides under the second m-pair's matmuls; AG2's hides
    under phase B's first half;
  - interleaved PSUM accumulation chains hide the per-matmul weight
    load (phase A: 2-bank pairs; phase B half 1: r-inner 5-bank ILP);
  - phase B half 2 runs r-OUTER so each psS[r] retires early and the
    zT = sum_r S_r o cinv_r combine overlaps the remaining matmuls,
    shrinking the serial BN tail.

Device strategy (per core c of 8, 512 node-rows each):
  Phase A: h'|root' = x[rows] @ [W'_0|..|W'_4|root'] (4096 x 450) in 16
           fine-grained load groups (x on sync-DGE, W' on scalar-DGE),
           m-tiles in 2 pair-passes (3 PSUM banks).  root' block stays
           in fp32 SBUF; h' -> bf16 SBUF -> h_cr[mg] DRAM (sync-DGE) ->
           192KB half-AllGather per m-pair.
  Phase B: per relation r: S_r[75, 512] accumulated over 32 src-tiles
           (stationary bf16 h' tiles, moving fp8 count tiles, 512-wide
           streams; 5 PSUM banks), split in two src-m halves to chase
           the two AllGathers.
  Phase C: per (m,r): PE-transpose zT -> [dst,75], + root' + bias';
           per-row BN (bn_stats/bn_aggr) + gamma/beta + ReLU.
"""
import numpy as np
import ml_dtypes

import concourse.bacc as bacc
import concourse.mybir as mybir
import concourse.tile as tile
from concourse.bass_utils import run_bass_kernel_spmd
from concourse.masks import make_identity

P = 128
NCORES = 8
N = 4096          # nodes
U = 2048          # users
R = 5             # relations
H0 = 500
H1 = 75
EPS = 1e-5

NL = N // NCORES              # 512 node rows per core
KB_A = N // P                 # 32 contraction tiles, phase A
GB_A = 16                     # phase-A load groups
KPG = KB_A // GB_A            # 2 kb per group
WCOL = R * H1 + H1            # 450 folded-weight columns
MB = NL // P                  # 4 M-tiles per core
KT_B = R * MB                 # 20 k-tiles per core-block in phase B
WARM = 3                      # at4 chunks prefetched during phase A

F32 = mybir.dt.float32
BF16 = mybir.dt.bfloat16
FP8 = mybir.dt.float8e4
NP_FP8 = ml_dtypes.float8_e4m3

# test hooks
TRACE = False
LAST_RESULTS = None
_NC_CACHE = None


def _build():
    nc = bacc.Bacc("TRN2", target_bir_lowering=False, debug=False,
                   num_devices=NCORES)

    # host-swizzled inputs; layouts noted as [partition, free...]
    # x4[p, kb*NL + m] = x[coreRows m][i = kb*128+p]
    x4_d = nc.dram_tensor("x4", [P, KB_A * NL], BF16, kind="ExternalInput")
    # w4[p, kb*WCOL + j] = Wall'[kb*128+p, j]
    w4_d = nc.dram_tensor("w4", [P, KB_A * WCOL], BF16, kind="ExternalInput")
    # at4[p, (h, cb, r, mk2)*NL + d] = Mcnt[(r, src), myDst d];
    # src = cb*512 + (h*2+mk2)*128 + p  (fp8 counts, src-m-half major)
    at4_d = nc.dram_tensor("at4", [P, NCORES * KT_B * NL], FP8,
                           kind="ExternalInput")
    # cinvT[0, r*NL + d] = 1 / max(cnt[dst = d @ core, r], 1)
    cinvT_d = nc.dram_tensor("cinvT", [1, R * NL], BF16,
                             kind="ExternalInput")
    biasb_d = nc.dram_tensor("biasb", [P, H1], F32, kind="ExternalInput")
    gamma_d = nc.dram_tensor("gamma", [P, MB], F32, kind="ExternalInput")
    beta_d = nc.dram_tensor("beta", [P, MB], F32, kind="ExternalInput")
    out_d = nc.dram_tensor("out", [NL, H1], F32, kind="ExternalOutput")

    HHW = 2 * R * H1              # 750 h'-columns per AllGather half
    HB = R * 2 * NL               # at4 columns per (half, cb) chunk
    with tile.TileContext(nc) as tc:
        with (
            tc.tile_pool(name="big", bufs=1) as big,
            tc.tile_pool(name="xtp", bufs=GB_A) as xtp,
            tc.tile_pool(name="wtp", bufs=GB_A) as wtp,
            tc.tile_pool(name="io", bufs=4) as iop,
            tc.tile_pool(name="hhp", bufs=2 * NCORES) as hhp,
            tc.tile_pool(name="atp", bufs=2) as atp,
            tc.tile_pool(name="persist", bufs=4) as pp,
            tc.tile_pool(name="stp", bufs=5) as stp,
            tc.tile_pool(name="zp", bufs=2) as zp,
            tc.tile_pool(name="bn", bufs=4) as bnp,
            tc.tile_pool(name="psA", bufs=3, space="PSUM") as psa,
            tc.tile_pool(name="psB", bufs=5, space="PSUM") as psb,
            tc.tile_pool(name="dram", bufs=1, space="DRAM") as dramp,
        ):
            # small setup tiles first (cheap; keeps them off the tail)
            ones75 = big.tile([1, H1], BF16, tag="ones")
            nc.vector.memset(ones75, 1.0)
            eps_t = big.tile([P, 1], F32, tag="eps")
            nc.vector.memset(eps_t, EPS)
            ident = big.tile([P, P], F32, tag="ident")
            make_identity(nc, ident)
            cinvT = big.tile([1, R * NL], BF16, tag="cinvT")
            nc.scalar.dma_start(out=cinvT, in_=cinvT_d[:, :])
            biasb = big.tile([P, H1], F32, tag="bias")
            nc.scalar.dma_start(out=biasb, in_=biasb_d[:, :])
            gam = big.tile([P, MB], F32, tag="gam")
            nc.scalar.dma_start(out=gam, in_=gamma_d[:, :])
            bet = big.tile([P, MB], F32, tag="bet")
            nc.scalar.dma_start(out=bet, in_=beta_d[:, :])

            # PE warm-up: the HAM clock gate holds the array at 1.2GHz
            # until ~3.4us of sustained activity; burn that window on
            # dummy matmuls while the first x/W groups stream in, so
            # phase A runs at 2.4GHz from its first instruction
            wrm = big.tile([P, P], BF16, tag="wrm")
            nc.vector.memset(wrm, 0.0)
            psW = psa.tile([P, P], F32, tag="psA", name="psW")
            for i in range(36):
                nc.tensor.matmul(psW, wrm, wrm, start=True, stop=True)

            # CC engage: each core's collectives core polls its trigger
            # queue on a ~30us period, and the FIRST collective's mesh-
            # begin waits for the slowest of the participating cores'
            # polls -- a 40-70us lottery that dominated run-to-run
            # variance.  This dummy 128B PAIRWISE AllGather fires at t~10
            # (content irrelevant): 2-rank groups sync after max-of-2
            # polls instead of max-of-8, so it retires early and leaves
            # every CC hot; the real all-8 AllGathers then begin
            # immediately on arrival and run back-to-back.
            cc0_in = dramp.tile([1, P], FP8, tag="cc0i")
            cc0_out = dramp.tile([2, P], FP8, tag="cc0o")
            with tc.high_priority():
                nc.gpsimd.collective_compute(
                    "AllGather",
                    mybir.AluOpType.bypass,
                    replica_groups=[[2 * i, 2 * i + 1]
                                    for i in range(NCORES // 2)],
                    ins=[cc0_in[:, :]],
                    outs=[cc0_out[:, :]],
                )

            # ---------------- Phase A: h' = x_rows @ Wall' ----------------
            xg, wg = [], []
            for g in range(GB_A):
                xt = xtp.tile([P, KPG, NL], BF16, tag="xt", name=f"xt_{g}")
                nc.sync.dma_start(
                    out=xt, in_=x4_d[:, g * KPG * NL:(g + 1) * KPG * NL])
                xg.append(xt)
                wt = wtp.tile([P, KPG, WCOL], BF16, tag="wt", name=f"wt_{g}")
                nc.scalar.dma_start(
                    out=wt, in_=w4_d[:, g * KPG * WCOL:(g + 1) * KPG * WCOL])
                wg.append(wt)

            # at4 rides the software-DGE queue (exempt from the
            # collective's hardware-DMA drain barrier) in 16 chunks, each
            # WAW-gated (1-elem vector pre-write dep on hb16[0]) so
            # transfers start only once x/W are nearly done --
            # unthrottled at4 starves phase A's input streams.  Chunking
            # keeps the Q7 sequencer interruptible: a SWDGE dma_start
            # occupies it for the whole transfer, and one 10MB dma would
            # push the AllGather triggers out by ~30us.
            aa = [atp.tile([P, NCORES, R * 2, NL], FP8, tag="aa",
                           name=f"aa{h}") for h in range(2)]

            # h_cr[h][p, m2*375 + r*75 + j] = h'[(2h+m2)*128+p, r*75+j]
            h_cr = [dramp.tile([P, HHW], BF16, tag=f"h_c{h}",
                               name=f"h_cr{h}") for h in range(2)]
            h_ar = [dramp.tile([NCORES * P, HHW], BF16, tag=f"h_a{h}",
                               addr_space="Shared", name=f"h_ar{h}")
                    for h in range(2)]

            rootf, hb16 = [], []
            for mg in range(2):          # m-pairs: 2-bank ILP, 3 psA bufs
                ps_m = [psa.tile([P, WCOL], F32, tag="psA",
                                 name=f"psA_{mg}_{mi}") for mi in range(2)]
                for g in range(GB_A):
                    for kb in range(KPG):
                        for mi in range(2):
                            m = mg * 2 + mi
                            nc.tensor.matmul(
                                ps_m[mi],
                                xg[g][:, kb, m * P:(m + 1) * P],
                                wg[g][:, kb, :],
                                start=(g == 0 and kb == 0),
                                stop=(g == GB_A - 1 and kb == KPG - 1),
                            )
                for mi in range(2):
                    m = mg * 2 + mi
                    rf = pp.tile([P, H1], F32, tag="rootf", name=f"rootf_{m}")
                    nc.vector.tensor_copy(out=rf, in_=ps_m[mi][:, R * H1:])
                    rootf.append(rf)
                    hb = iop.tile([P, R * H1], BF16, tag="hout",
                                  name=f"hout_{m}")
                    nc.vector.tensor_copy(out=hb, in_=ps_m[mi][:, :R * H1])
                    hb16.append(hb)
                    # hardware-DGE store: drains in <1us, so the trigger
                    # below fires as soon as this m-pair's h' lands
                    nc.sync.dma_start(
                        out=h_cr[mg][:, mi * R * H1:(mi + 1) * R * H1],
                        in_=hb)
                # half-AllGather per m-pair: AG1's latency+wire hide
                # under phase A's second m-pair; AG2 queues on the CC
                # core behind AG1 and its wire hides under phase B's
                # first half.  high_priority slots each trigger between
                # at4 chunks on the Q7 the moment its h_cr dep fires.
                with tc.high_priority():
                    nc.gpsimd.collective_compute(
                        "AllGather",
                        mybir.AluOpType.bypass,
                        replica_groups=[list(range(NCORES))],
                        ins=[h_cr[mg][:, :]],
                        outs=[h_ar[mg][:, :]],
                    )
                # at4 half-h chunks are gated on a 1-elem DRAM read-back
                # of h_cr[h]: their ready-time then TIES with trigger
                # h's, and high_priority wins the tie -- otherwise the
                # scheduler slots transfer-rate-paced chunk emissions
                # before the trigger on the Q7 and delays it ~25us
                gate = big.tile([1, 1], BF16, tag=f"gate{mg}",
                                name=f"gate{mg}")
                nc.gpsimd.dma_start(out=gate, in_=h_cr[mg][0:1, 0:1])
                for cb in range(NCORES):
                    nc.vector.tensor_copy(
                        out=aa[mg][0:1, cb, 0, 0:1], in_=gate)
                for cb in range(NCORES):
                    base = (mg * NCORES + cb) * HB
                    nc.gpsimd.dma_start(
                        out=aa[mg][:, cb],
                        in_=at4_d[:, base:base + HB])

            # rank-1 broadcast rows B_r = ones[75] x cinv_r[512]; runs on
            # the PE between phase A and B (no phase-B dependency)
            brs = []
            for r in range(R):
                br = psa.tile([H1, NL], F32, tag="psA", name=f"br_{r}")
                nc.tensor.matmul(br, ones75, cinvT[:, r * NL:(r + 1) * NL],
                                 start=True, stop=True)
                bs = stp.tile([H1, NL], F32, tag="sT", name=f"brs_{r}")
                nc.vector.tensor_copy(out=bs, in_=br)
                brs.append(bs)

            # ------- Phase B: S_r = sum_s h'_r-tile.T @ Mcnt-tile ---------
            # half 1 (src m-tiles 0,1 of every core) follows AG1; hh rows
            # stream on the idle sync hardware queue at full rate
            hh = [[None] * NCORES for _ in range(2)]
            for cb in range(NCORES):
                t = hhp.tile([P, HHW], BF16, tag="hh", name=f"hh0_{cb}")
                nc.sync.dma_start(out=t, in_=h_ar[0][cb * P:(cb + 1) * P, :])
                hh[0][cb] = t
            psS = [psb.tile([H1, NL], F32, tag="psB", name=f"psB_{r}")
                   for r in range(R)]
            for cb in range(NCORES):
                for mk2 in range(2):
                    for r in range(R):
                        nc.tensor.matmul(
                            psS[r],
                            hh[0][cb][:, mk2 * R * H1 + r * H1:
                                      mk2 * R * H1 + (r + 1) * H1],
                            aa[0][:, cb, r * 2 + mk2, :],
                            start=(cb == 0 and mk2 == 0),
                            stop=False,
                        )

            # half 2 follows AG2, r-OUTER: each psS[r] retires after its
            # sweep so the zT = sum_r S_r o cinv_r combine overlaps the
            # remaining matmuls instead of serializing after phase B
            for cb in range(NCORES):
                t = hhp.tile([P, HHW], BF16, tag="hh", name=f"hh1_{cb}")
                nc.sync.dma_start(out=t, in_=h_ar[1][cb * P:(cb + 1) * P, :])
                hh[1][cb] = t
            zT = pp.tile([H1, NL], F32, tag="zT")
            for r in range(R):
                for cb in range(NCORES):
                    for mk2 in range(2):
                        nc.tensor.matmul(
                            psS[r],
                            hh[1][cb][:, mk2 * R * H1 + r * H1:
                                      mk2 * R * H1 + (r + 1) * H1],
                            aa[1][:, cb, r * 2 + mk2, :],
                            start=False,
                            stop=(cb == NCORES - 1 and mk2 == 1),
                        )
                # psS[r] complete -> fold into zT while r+1 accumulates
                if r == 0:
                    nc.vector.tensor_mul(out=zT, in0=psS[0], in1=brs[0])
                else:
                    ztmp = zp.tile([H1, NL], F32, tag="ztmp",
                                   name=f"ztmp_{r}")
                    nc.vector.tensor_mul(out=ztmp, in0=psS[r], in1=brs[r])
                    nc.vector.tensor_add(out=zT, in0=zT, in1=ztmp)

            # ---------------- Phase C: combine -> BN -> ReLU --------------
            for m in range(MB):
                pt = psa.tile([P, H1], F32, tag="psA", name=f"pt_{m}")
                nc.tensor.transpose(
                    pt, zT[:, m * P:(m + 1) * P], ident[:H1, :H1]
                )
                acc = bnp.tile([P, H1], F32, tag="acc", name=f"acc_{m}")
                nc.vector.tensor_add(out=acc, in0=pt, in1=rootf[m])
                nc.vector.tensor_add(out=acc, in0=acc, in1=biasb)

                stats = bnp.tile([P, 6], F32, tag="stats")
                nc.vector.bn_stats(out=stats, in_=acc)
                mv = bnp.tile([P, 2], F32, tag="mv")
                nc.vector.bn_aggr(out=mv, in_=stats)
                rstd = bnp.tile([P, 1], F32, tag="rstd")
                nc.scalar.activation(
                    out=rstd, in_=mv[:, 1:2],
                    func=mybir.ActivationFunctionType.Sqrt,
                    bias=eps_t, scale=1.0,
                )
                nc.vector.reciprocal(out=rstd, in_=rstd)
                g2 = bnp.tile([P, 1], F32, tag="g2")
                nc.vector.tensor_mul(out=g2, in0=rstd, in1=gam[:, m:m + 1])
                zt = bnp.tile([P, H1], F32, tag="zt")
                nc.vector.tensor_scalar(
                    out=zt, in0=acc,
                    scalar1=mv[:, 0:1], scalar2=g2,
                    op0=mybir.AluOpType.subtract, op1=mybir.AluOpType.mult,
                )
                nc.scalar.activation(
                    out=zt, in_=zt,
                    func=mybir.ActivationFunctionType.Relu,
                    bias=bet[:, m:m + 1], scale=1.0,
                )
                nc.scalar.dma_start(out=out_d[m * P:(m + 1) * P, :], in_=zt)

    nc.finalize()
    return nc


def _get_nc():
    global _NC_CACHE
    if _NC_CACHE is None:
        _NC_CACHE = _build()
    return _NC_CACHE


def kernel(**inputs) -> np.ndarray:
    global LAST_RESULTS
    x = np.asarray(inputs["x"], dtype=np.float32)
    basis = np.asarray(inputs["basis"], dtype=np.float32)
    comp = np.asarray(inputs["comp"], dtype=np.float32)
    root = np.asarray(inputs["root"], dtype=np.float32)
    bias_rgcn = np.asarray(inputs["bias_rgcn"], dtype=np.float32)
    fc_w = np.asarray(inputs["fc_w"], dtype=np.float32)
    bn_gamma_u = np.asarray(inputs["bn_gamma_u"], dtype=np.float32)
    bn_beta_u = np.asarray(inputs["bn_beta_u"], dtype=np.float32)
    bn_gamma_i = np.asarray(inputs["bn_gamma_i"], dtype=np.float32)
    bn_beta_i = np.asarray(inputs["bn_beta_i"], dtype=np.float32)
    edge_index = np.asarray(inputs["edge_index"]).astype(np.int64)
    edge_type = np.asarray(inputs["edge_type"]).astype(np.int64)

    src, dst = edge_index[0], edge_index[1]
    et = edge_type

    # Wall' = [W_r @ fc_w.T for r | root @ fc_w.T]  (fold the Dense layer)
    W = np.tensordot(comp, basis, axes=([1], [0]))          # [R, N, H0]
    Wp = np.einsum("rio,jo->rij", W, fc_w, optimize=True)   # [R, N, H1]
    wall = np.empty((N, WCOL), dtype=np.float32)
    wall[:, :R * H1] = Wp.transpose(1, 0, 2).reshape(N, R * H1)
    wall[:, R * H1:] = root @ fc_w.T
    w4 = np.ascontiguousarray(
        wall.astype(ml_dtypes.bfloat16)
        .reshape(KB_A, P, WCOL)                 # [kb, p, j]
        .transpose(1, 0, 2)                     # [p, kb, j]
        .reshape(P, KB_A * WCOL))

    xT16 = np.ascontiguousarray(x.T).astype(ml_dtypes.bfloat16)
    # x4[p, kb*NL + m] = x.T[kb*128+p, m@core]  (per-core slice below)
    x4_full = (xT16.reshape(KB_A, P, N)         # [kb, p, s]
               .transpose(1, 0, 2))             # [p, kb, s]

    # integer multiplicity matrix Mcnt[(r, src), dst] (exact in fp8e4m3)
    lin = (et * N + src) * np.int64(N) + dst
    cntmat = np.bincount(lin, minlength=R * N * N)
    assert cntmat.max() <= 16, "edge multiplicity too large for fp8 counts"
    atw = cntmat.astype(NP_FP8).reshape(R, NCORES, MB, P, N)

    # per-(dst, r) inverse counts
    cnt = np.bincount(dst * R + et, minlength=N * R).astype(np.float64)
    cinv_full = (1.0 / np.maximum(cnt, 1.0)).astype(np.float32).reshape(N, R)

    biasb = np.ascontiguousarray(
        np.broadcast_to(bias_rgcn @ fc_w.T, (P, H1)), dtype=np.float32)
    gamma_all = np.concatenate([bn_gamma_u, bn_gamma_i])
    beta_all = np.concatenate([bn_beta_u, bn_beta_i])

    in_maps = []
    for c in range(NCORES):
        sl = slice(c * NL, (c + 1) * NL)
        atc = atw[:, :, :, :, sl]               # [r, cb, mk, p, d]
        at4 = (atc.reshape(R, NCORES, 2, 2, P, NL)  # mk -> (h, mk2)
               .transpose(4, 2, 1, 0, 3, 5)         # [p, h, cb, r, mk2, d]
               .reshape(P, NCORES * KT_B * NL))
        in_maps.append({
            "x4": np.ascontiguousarray(
                x4_full[:, :, sl]).reshape(P, KB_A * NL),
            "w4": w4,
            "at4": np.ascontiguousarray(at4),
            "cinvT": np.ascontiguousarray(
                cinv_full[sl].T).astype(ml_dtypes.bfloat16).reshape(1, R * NL),
            "biasb": biasb,
            "gamma": np.ascontiguousarray(gamma_all[sl].reshape(MB, P).T),
            "beta": np.ascontiguousarray(beta_all[sl].reshape(MB, P).T),
        })

    nc = _get_nc()
    import os as _os
    _tc = (list(range(NCORES))
           if _os.environ.get("KTRACE_ALL") == "1" else None)
    res = run_bass_kernel_spmd(
        nc, in_maps, core_ids=list(range(NCORES)), trace=TRACE,
        trace_cores=_tc,
    )
    LAST_RESULTS = res

    z = np.concatenate([res.results[c]["out"] for c in range(NCORES)], axis=0)
    return np.stack([z[:U], z[U:]], axis=0)



# revision 27
# speedup vs baseline: 1.0417x; 1.0417x over previous
"""GCEncoder (RGCN basis-decomposition conv + mean aggregation + Dense/BN/ReLU)
as a Bass/Tile kernel on 8 Trainium2 NeuronCores.

Math (reference):
  W[r]  = sum_b comp[r,b] * basis[b]                    [R, N, H0]
  h[r]  = x @ W[r]                                      [R, N, H0]
  agg[d] = sum_r (1/cnt[d,r]) * sum_{e: dst=d, type=r} h[r, src_e]
  feats = agg + x @ root + bias
  z     = feats @ fc_w.T ; per-row batchnorm over H1 + gamma/beta + relu
  out   = (z[:U], z[U:]) stacked -> [2, U, H1]

Everything before the BN is linear in the H0 axis, so fc_w is folded into
the weights on the host: W'[r] = W[r] @ fc_w.T (4096 x 75), root' =
root @ fc_w.T, bias' = bias @ fc_w.T.  The device only moves 75-wide
features (~6.7x fewer matmul FLOPs than the unfolded form):

  z[d] = sum_r (1/cnt[d,r]) * (Mcnt_r[d,:] @ h'_r) + x[d] @ root' + bias'

with Mcnt_r the integer edge-multiplicity matrix (exact in fp8e4m3, half
the HBM bytes of a bf16 weighted adjacency) and h'_r = x @ W'_r.

Scheduling facts learned from perfetto traces on this stack:
  - the collective trigger waits for ALL in-flight hardware-DGE DMA to
    drain, so bulk at4 streaming rides the software-DGE (gpsimd) queue,
    which is exempt; the tiny h_cr stores ride the hardware queue so
    they drain fast and never queue behind at4 (the single-AG baseline
    lost ~35us to exactly that);
  - a collective costs ~11us trigger->mesh plus wire time; TWO half
    AllGathers pipeline: AG1 (m-tiles 0,1) triggers mid-phase-A and its
    wire time hides under the second m-pair's matmuls; AG2's hides
    under phase B's first half;
  - interleaved PSUM accumulation chains hide the per-matmul weight
    load (phase A: 2-bank pairs; phase B half 1: r-inner 5-bank ILP);
  - phase B half 2 runs r-OUTER so each psS[r] retires early and the
    zT = sum_r S_r o cinv_r combine overlaps the remaining matmuls,
    shrinking the serial BN tail.

Device strategy (per core c of 8, 512 node-rows each):
  Phase A: h'|root' = x[rows] @ [W'_0|..|W'_4|root'] (4096 x 450) in 16
           fine-grained load groups (x on sync-DGE, W' on scalar-DGE),
           m-tiles in 2 pair-passes (3 PSUM banks).  root' block stays
           in fp32 SBUF; h' -> bf16 SBUF -> h_cr[mg] DRAM (sync-DGE) ->
           192KB half-AllGather per m-pair.
  Phase B: per relation r: S_r[75, 512] accumulated over 32 src-tiles
           (stationary bf16 h' tiles, moving fp8 count tiles, 512-wide
           streams; 5 PSUM banks), split in two src-m halves to chase
           the two AllGathers.
  Phase C: per (m,r): PE-transpose zT -> [dst,75], + root' + bias';
           per-row BN (bn_stats/bn_aggr) + gamma/beta + ReLU.
"""
import numpy as np
import ml_dtypes

import concourse.bacc as bacc
import concourse.mybir as mybir
import concourse.tile as tile
from concourse.bass_utils import run_bass_kernel_spmd
from concourse.masks import make_identity

P = 128
NCORES = 8
N = 4096          # nodes
U = 2048          # users
R = 5             # relations
H0 = 500
H1 = 75
EPS = 1e-5

NL = N // NCORES              # 512 node rows per core
KB_A = N // P                 # 32 contraction tiles, phase A
GB_A = 16                     # phase-A load groups
KPG = KB_A // GB_A            # 2 kb per group
WCOL = R * H1 + H1            # 450 folded-weight columns
MB = NL // P                  # 4 M-tiles per core
KT_B = R * MB                 # 20 k-tiles per core-block in phase B
WARM = 3                      # at4 chunks prefetched during phase A

F32 = mybir.dt.float32
BF16 = mybir.dt.bfloat16
FP8 = mybir.dt.float8e4
NP_FP8 = ml_dtypes.float8_e4m3

# test hooks
TRACE = False
LAST_RESULTS = None
_NC_CACHE = None


def _build():
    nc = bacc.Bacc("TRN2", target_bir_lowering=False, debug=False,
                   num_devices=NCORES)

    # host-swizzled inputs; layouts noted as [partition, free...]
    # x4[p, kb*NL + m] = x[coreRows m][i = kb*128+p]
    x4_d = nc.dram_tensor("x4", [P, KB_A * NL], BF16, kind="ExternalInput")
    # w4[p, kb*WCOL + j] = Wall'[kb*128+p, j]
    w4_d = nc.dram_tensor("w4", [P, KB_A * WCOL], BF16, kind="ExternalInput")
    # at4[p, (h, cb, r, mk2)*NL + d] = Mcnt[(r, src), myDst d];
    # src = cb*512 + (h*2+mk2)*128 + p  (fp8 counts, src-m-half major)
    at4_d = nc.dram_tensor("at4", [P, NCORES * KT_B * NL], FP8,
                           kind="ExternalInput")
    # cinvT[0, r*NL + d] = 1 / max(cnt[dst = d @ core, r], 1)
    cinvT_d = nc.dram_tensor("cinvT", [1, R * NL], BF16,
                             kind="ExternalInput")
    biasb_d = nc.dram_tensor("biasb", [P, H1], F32, kind="ExternalInput")
    gamma_d = nc.dram_tensor("gamma", [P, MB], F32, kind="ExternalInput")
    beta_d = nc.dram_tensor("beta", [P, MB], F32, kind="ExternalInput")
    out_d = nc.dram_tensor("out", [NL, H1], F32, kind="ExternalOutput")

    HHW = 2 * R * H1              # 750 h'-columns per AllGather half
    HB = R * 2 * NL               # at4 columns per (half, cb) chunk
    with tile.TileContext(nc) as tc:
        with (
            tc.tile_pool(name="big", bufs=1) as big,
            tc.tile_pool(name="xtp", bufs=GB_A) as xtp,
            tc.tile_pool(name="wtp", bufs=GB_A) as wtp,
            tc.tile_pool(name="io", bufs=4) as iop,
            tc.tile_pool(name="hhp", bufs=2 * NCORES) as hhp,
            tc.tile_pool(name="atp", bufs=2) as atp,
            tc.tile_pool(name="persist", bufs=4) as pp,
            tc.tile_pool(name="stp", bufs=5) as stp,
            tc.tile_pool(name="zp", bufs=2) as zp,
            tc.tile_pool(name="bn", bufs=4) as bnp,
            tc.tile_pool(name="psA", bufs=3, space="PSUM") as psa,
            tc.tile_pool(name="psB", bufs=5, space="PSUM") as psb,
            tc.tile_pool(name="dram", bufs=1, space="DRAM") as dramp,
        ):
            # small setup tiles first (cheap; keeps them off the tail)
            ones75 = big.tile([1, H1], BF16, tag="ones")
            nc.vector.memset(ones75, 1.0)
            eps_t = big.tile([P, 1], F32, tag="eps")
            nc.vector.memset(eps_t, EPS)
            ident = big.tile([P, P], F32, tag="ident")
            make_identity(nc, ident)
            cinvT = big.tile([1, R * NL], BF16, tag="cinvT")
            nc.scalar.dma_start(out=cinvT, in_=cinvT_d[:, :])
            biasb = big.tile([P, H1], F32, tag="bias")
            nc.scalar.dma_start(out=biasb, in_=biasb_d[:, :])
            gam = big.tile([P, MB], F32, tag="gam")
            nc.scalar.dma_start(out=gam, in_=gamma_d[:, :])
            bet = big.tile([P, MB], F32, tag="bet")
            nc.scalar.dma_start(out=bet, in_=beta_d[:, :])

            # PE warm-up: the HAM clock gate holds the array at 1.2GHz
            # until ~3.4us of sustained activity; burn that window on
            # dummy matmuls while the first x/W groups stream in, so
            # phase A runs at 2.4GHz from its first instruction
            wrm = big.tile([P, P], BF16, tag="wrm")
            nc.vector.memset(wrm, 0.0)
            psW = psa.tile([P, P], F32, tag="psA", name="psW")
            for i in range(36):
                nc.tensor.matmul(psW, wrm, wrm, start=True, stop=True)

            # ---------------- Phase A: h' = x_rows @ Wall' ----------------
            xg, wg = [], []
            for g in range(GB_A):
                xt = xtp.tile([P, KPG, NL], BF16, tag="xt", name=f"xt_{g}")
                nc.sync.dma_start(
                    out=xt, in_=x4_d[:, g * KPG * NL:(g + 1) * KPG * NL])
                xg.append(xt)
                wt = wtp.tile([P, KPG, WCOL], BF16, tag="wt", name=f"wt_{g}")
                nc.scalar.dma_start(
                    out=wt, in_=w4_d[:, g * KPG * WCOL:(g + 1) * KPG * WCOL])
                wg.append(wt)

            # at4 rides the software-DGE queue (exempt from the
            # collective's hardware-DMA drain barrier) in 16 chunks, each
            # WAW-gated (1-elem vector pre-write dep on hb16[0]) so
            # transfers start only once x/W are nearly done --
            # unthrottled at4 starves phase A's input streams.  Chunking
            # keeps the Q7 sequencer interruptible: a SWDGE dma_start
            # occupies it for the whole transfer, and one 10MB dma would
            # push the AllGather triggers out by ~30us.
            aa = [atp.tile([P, NCORES, R * 2, NL], FP8, tag="aa",
                           name=f"aa{h}") for h in range(2)]

            # h_cr[h][p, m2*375 + r*75 + j] = h'[(2h+m2)*128+p, r*75+j]
            h_cr = [dramp.tile([P, HHW], BF16, tag=f"h_c{h}",
                               name=f"h_cr{h}") for h in range(2)]
            h_ar = [dramp.tile([NCORES * P, HHW], BF16, tag=f"h_a{h}",
                               addr_space="Shared", name=f"h_ar{h}")
                    for h in range(2)]

            rootf, hb16 = [], []
            for mg in range(2):          # m-pairs: 2-bank ILP, 3 psA bufs
                ps_m = [psa.tile([P, WCOL], F32, tag="psA",
                                 name=f"psA_{mg}_{mi}") for mi in range(2)]
                for g in range(GB_A):
                    for kb in range(KPG):
                        for mi in range(2):
                            m = mg * 2 + mi
                            nc.tensor.matmul(
                                ps_m[mi],
                                xg[g][:, kb, m * P:(m + 1) * P],
                                wg[g][:, kb, :],
                                start=(g == 0 and kb == 0),
                                stop=(g == GB_A - 1 and kb == KPG - 1),
                            )
                for mi in range(2):
                    m = mg * 2 + mi
                    rf = pp.tile([P, H1], F32, tag="rootf", name=f"rootf_{m}")
                    nc.vector.tensor_copy(out=rf, in_=ps_m[mi][:, R * H1:])
                    rootf.append(rf)
                    hb = iop.tile([P, R * H1], BF16, tag="hout",
                                  name=f"hout_{m}")
                    nc.vector.tensor_copy(out=hb, in_=ps_m[mi][:, :R * H1])
                    hb16.append(hb)
                    # hardware-DGE store: drains in <1us, so the trigger
                    # below fires as soon as this m-pair's h' lands
                    nc.sync.dma_start(
                        out=h_cr[mg][:, mi * R * H1:(mi + 1) * R * H1],
                        in_=hb)
                # half-AllGather per m-pair: AG1's latency+wire hide
                # under phase A's second m-pair; AG2 queues on the CC
                # core behind AG1 and its wire hides under phase B's
                # first half.  high_priority slots each trigger between
                # at4 chunks on the Q7 the moment its h_cr dep fires.
                with tc.high_priority():
                    nc.gpsimd.collective_compute(
                        "AllGather",
                        mybir.AluOpType.bypass,
                        replica_groups=[list(range(NCORES))],
                        ins=[h_cr[mg][:, :]],
                        outs=[h_ar[mg][:, :]],
                    )
                if mg == 0:
                    # at4 in 16 chunks on the software-DGE queue, each
                    # WAW-gated (1-elem vector pre-write dep on hb16[0])
                    # so transfers start only once x/W are nearly done --
                    # unthrottled at4 starves phase A's input streams
                    for h in range(2):
                        for cb in range(NCORES):
                            nc.vector.tensor_copy(
                                out=aa[h][0:1, cb, 0, 0:1],
                                in_=hb16[0][0:1, 0:1])
                    for h in range(2):
                        for cb in range(NCORES):
                            base = (h * NCORES + cb) * HB
                            nc.gpsimd.dma_start(
                                out=aa[h][:, cb],
                                in_=at4_d[:, base:base + HB])

            # rank-1 broadcast rows B_r = ones[75] x cinv_r[512]; runs on
            # the PE between phase A and B (no phase-B dependency)
            brs = []
            for r in range(R):
                br = psa.tile([H1, NL], F32, tag="psA", name=f"br_{r}")
                nc.tensor.matmul(br, ones75, cinvT[:, r * NL:(r + 1) * NL],
                                 start=True, stop=True)
                bs = stp.tile([H1, NL], F32, tag="sT", name=f"brs_{r}")
                nc.vector.tensor_copy(out=bs, in_=br)
                brs.append(bs)

            # ------- Phase B: S_r = sum_s h'_r-tile.T @ Mcnt-tile ---------
            # half 1 (src m-tiles 0,1 of every core) follows AG1; hh rows
            # stream on the idle sync hardware queue at full rate
            hh = [[None] * NCORES for _ in range(2)]
            for cb in range(NCORES):
                t = hhp.tile([P, HHW], BF16, tag="hh", name=f"hh0_{cb}")
                nc.sync.dma_start(out=t, in_=h_ar[0][cb * P:(cb + 1) * P, :])
                hh[0][cb] = t
            psS = [psb.tile([H1, NL], F32, tag="psB", name=f"psB_{r}")
                   for r in range(R)]
            for cb in range(NCORES):
                for mk2 in range(2):
                    for r in range(R):
                        nc.tensor.matmul(
                            psS[r],
                            hh[0][cb][:, mk2 * R * H1 + r * H1:
                                      mk2 * R * H1 + (r + 1) * H1],
                            aa[0][:, cb, r * 2 + mk2, :],
                            start=(cb == 0 and mk2 == 0),
                            stop=False,
                        )

            # half 2 follows AG2, r-OUTER: each psS[r] retires after its
            # sweep so the zT = sum_r S_r o cinv_r combine overlaps the
            # remaining matmuls instead of serializing after phase B
            for cb in range(NCORES):
                t = hhp.tile([P, HHW], BF16, tag="hh", name=f"hh1_{cb}")
                nc.sync.dma_start(out=t, in_=h_ar[1][cb * P:(cb + 1) * P, :])
                hh[1][cb] = t
            zT = pp.tile([H1, NL], F32, tag="zT")
            for r in range(R):
                for cb in range(NCORES):
                    for mk2 in range(2):
                        nc.tensor.matmul(
                            psS[r],
                            hh[1][cb][:, mk2 * R * H1 + r * H1:
                                      mk2 * R * H1 + (r + 1) * H1],
                            aa[1][:, cb, r * 2 + mk2, :],
                            start=False,
                            stop=(cb == NCORES - 1 and mk2 == 1),
                        )
                # psS[r] complete -> fold into zT while r+1 accumulates
                if r == 0:
                    nc.vector.tensor_mul(out=zT, in0=psS[0], in1=brs[0])
                else:
                    ztmp = zp.tile([H1, NL], F32, tag="ztmp",
                                   name=f"ztmp_{r}")
                    nc.vector.tensor_mul(out=ztmp, in0=psS[r], in1=brs[r])
                    nc.vector.tensor_add(out=zT, in0=zT, in1=ztmp)

            # ---------------- Phase C: combine -> BN -> ReLU --------------
            for m in range(MB):
                pt = psa.tile([P, H1], F32, tag="psA", name=f"pt_{m}")
                nc.tensor.transpose(
                    pt, zT[:, m * P:(m + 1) * P], ident[:H1, :H1]
                )
                acc = bnp.tile([P, H1], F32, tag="acc", name=f"acc_{m}")
                nc.vector.tensor_add(out=acc, in0=pt, in1=rootf[m])
                nc.vector.tensor_add(out=acc, in0=acc, in1=biasb)

                stats = bnp.tile([P, 6], F32, tag="stats")
                nc.vector.bn_stats(out=stats, in_=acc)
                mv = bnp.tile([P, 2], F32, tag="mv")
                nc.vector.bn_aggr(out=mv, in_=stats)
                rstd = bnp.tile([P, 1], F32, tag="rstd")
                nc.scalar.activation(
                    out=rstd, in_=mv[:, 1:2],
                    func=mybir.ActivationFunctionType.Sqrt,
                    bias=eps_t, scale=1.0,
                )
                nc.vector.reciprocal(out=rstd, in_=rstd)
                g2 = bnp.tile([P, 1], F32, tag="g2")
                nc.vector.tensor_mul(out=g2, in0=rstd, in1=gam[:, m:m + 1])
                zt = bnp.tile([P, H1], F32, tag="zt")
                nc.vector.tensor_scalar(
                    out=zt, in0=acc,
                    scalar1=mv[:, 0:1], scalar2=g2,
                    op0=mybir.AluOpType.subtract, op1=mybir.AluOpType.mult,
                )
                nc.scalar.activation(
                    out=zt, in_=zt,
                    func=mybir.ActivationFunctionType.Relu,
                    bias=bet[:, m:m + 1], scale=1.0,
                )
                nc.scalar.dma_start(out=out_d[m * P:(m + 1) * P, :], in_=zt)

    nc.finalize()
    return nc


def _get_nc():
    global _NC_CACHE
    if _NC_CACHE is None:
        _NC_CACHE = _build()
    return _NC_CACHE


def kernel(**inputs) -> np.ndarray:
    global LAST_RESULTS
    x = np.asarray(inputs["x"], dtype=np.float32)
    basis = np.asarray(inputs["basis"], dtype=np.float32)
    comp = np.asarray(inputs["comp"], dtype=np.float32)
    root = np.asarray(inputs["root"], dtype=np.float32)
    bias_rgcn = np.asarray(inputs["bias_rgcn"], dtype=np.float32)
    fc_w = np.asarray(inputs["fc_w"], dtype=np.float32)
    bn_gamma_u = np.asarray(inputs["bn_gamma_u"], dtype=np.float32)
    bn_beta_u = np.asarray(inputs["bn_beta_u"], dtype=np.float32)
    bn_gamma_i = np.asarray(inputs["bn_gamma_i"], dtype=np.float32)
    bn_beta_i = np.asarray(inputs["bn_beta_i"], dtype=np.float32)
    edge_index = np.asarray(inputs["edge_index"]).astype(np.int64)
    edge_type = np.asarray(inputs["edge_type"]).astype(np.int64)

    src, dst = edge_index[0], edge_index[1]
    et = edge_type

    # Wall' = [W_r @ fc_w.T for r | root @ fc_w.T]  (fold the Dense layer)
    W = np.tensordot(comp, basis, axes=([1], [0]))          # [R, N, H0]
    Wp = np.einsum("rio,jo->rij", W, fc_w, optimize=True)   # [R, N, H1]
    wall = np.empty((N, WCOL), dtype=np.float32)
    wall[:, :R * H1] = Wp.transpose(1, 0, 2).reshape(N, R * H1)
    wall[:, R * H1:] = root @ fc_w.T
    w4 = np.ascontiguousarray(
        wall.astype(ml_dtypes.bfloat16)
        .reshape(KB_A, P, WCOL)                 # [kb, p, j]
        .transpose(1, 0, 2)                     # [p, kb, j]
        .reshape(P, KB_A * WCOL))

    xT16 = np.ascontiguousarray(x.T).astype(ml_dtypes.bfloat16)
    # x4[p, kb*NL + m] = x.T[kb*128+p, m@core]  (per-core slice below)
    x4_full = (xT16.reshape(KB_A, P, N)         # [kb, p, s]
               .transpose(1, 0, 2))             # [p, kb, s]

    # integer multiplicity matrix Mcnt[(r, src), dst] (exact in fp8e4m3)
    lin = (et * N + src) * np.int64(N) + dst
    cntmat = np.bincount(lin, minlength=R * N * N)
    assert cntmat.max() <= 16, "edge multiplicity too large for fp8 counts"
    atw = cntmat.astype(NP_FP8).reshape(R, NCORES, MB, P, N)

    # per-(dst, r) inverse counts
    cnt = np.bincount(dst * R + et, minlength=N * R).astype(np.float64)
    cinv_full = (1.0 / np.maximum(cnt, 1.0)).astype(np.float32).reshape(N, R)

    biasb = np.ascontiguousarray(
        np.broadcast_to(bias_rgcn @ fc_w.T, (P, H1)), dtype=np.float32)
    gamma_all = np.concatenate([bn_gamma_u, bn_gamma_i])
    beta_all = np.concatenate([bn_beta_u, bn_beta_i])

    in_maps = []
    for c in range(NCORES):
        sl = slice(c * NL, (c + 1) * NL)
        atc = atw[:, :, :, :, sl]               # [r, cb, mk, p, d]
        at4 = (atc.reshape(R, NCORES, 2, 2, P, NL)  # mk -> (h, mk2)
               .transpose(4, 2, 1, 0, 3, 5)         # [p, h, cb, r, mk2, d]
               .reshape(P, NCORES * KT_B * NL))
        in_maps.append({
            "x4": np.ascontiguousarray(
                x4_full[:, :, sl]).reshape(P, KB_A * NL),
            "w4": w4,
            "at4": np.ascontiguousarray(at4),
            "cinvT": np.ascontiguousarray(
                cinv_full[sl].T).astype(ml_dtypes.bfloat16).reshape(1, R * NL),
            "biasb": biasb,
            "gamma": np.ascontiguousarray(gamma_all[sl].reshape(MB, P).T),
            "beta": np.ascontiguousarray(beta_all[sl].reshape(MB, P).T),
        })

    nc = _get_nc()
    import os as _os
    _tc = (list(range(NCORES))
           if _os.environ.get("KTRACE_ALL") == "1" else None)
    res = run_bass_kernel_spmd(
        nc, in_maps, core_ids=list(range(NCORES)), trace=TRACE,
        trace_cores=_tc,
    )
    LAST_RESULTS = res

    z = np.concatenate([res.results[c]["out"] for c in range(NCORES)], axis=0)
    return np.stack([z[:U], z[U:]], axis=0)



# revision 28
# speedup vs baseline: 1.0692x; 1.0264x over previous
"""GCEncoder (RGCN basis-decomposition conv + mean aggregation + Dense/BN/ReLU)
as a Bass/Tile kernel on 8 Trainium2 NeuronCores.

Math (reference):
  W[r]  = sum_b comp[r,b] * basis[b]                    [R, N, H0]
  h[r]  = x @ W[r]                                      [R, N, H0]
  agg[d] = sum_r (1/cnt[d,r]) * sum_{e: dst=d, type=r} h[r, src_e]
  feats = agg + x @ root + bias
  z     = feats @ fc_w.T ; per-row batchnorm over H1 + gamma/beta + relu
  out   = (z[:U], z[U:]) stacked -> [2, U, H1]

Everything before the BN is linear in the H0 axis, so fc_w is folded into
the weights on the host: W'[r] = W[r] @ fc_w.T (4096 x 75), root' =
root @ fc_w.T, bias' = bias @ fc_w.T.  The device only moves 75-wide
features (~6.7x fewer matmul FLOPs than the unfolded form):

  z[d] = sum_r (1/cnt[d,r]) * (Mcnt_r[d,:] @ h'_r) + x[d] @ root' + bias'

with Mcnt_r the integer edge-multiplicity matrix (exact in fp8e4m3, half
the HBM bytes of a bf16 weighted adjacency) and h'_r = x @ W'_r.

Scheduling facts learned from perfetto traces on this stack:
  - the collective trigger waits for ALL in-flight hardware-DGE DMA to
    drain, so bulk at4 streaming rides the software-DGE (gpsimd) queue,
    which is exempt; the tiny h_cr stores ride the hardware queue so
    they drain fast and never queue behind at4 (the single-AG baseline
    lost ~35us to exactly that);
  - a collective costs ~11us trigger->mesh plus wire time; TWO half
    AllGathers pipeline: AG1 (m-tiles 0,1) triggers mid-phase-A and its
    wire time hides under the second m-pair's matmuls; AG2's hides
    under phase B's first half;
  - interleaved PSUM accumulation chains hide the per-matmul weight
    load (phase A: 2-bank pairs; phase B half 1: r-inner 5-bank ILP);
  - phase B half 2 runs r-OUTER so each psS[r] retires early and the
    zT = sum_r S_r o cinv_r combine overlaps the remaining matmuls,
    shrinking the serial BN tail.

Device strategy (per core c of 8, 512 node-rows each):
  Phase A: h'|root' = x[rows] @ [W'_0|..|W'_4|root'] (4096 x 450) in 16
           fine-grained load groups (x on sync-DGE, W' on scalar-DGE),
           m-tiles in 2 pair-passes (3 PSUM banks).  root' block stays
           in fp32 SBUF; h' -> bf16 SBUF -> h_cr[mg] DRAM (sync-DGE) ->
           192KB half-AllGather per m-pair.
  Phase B: per relation r: S_r[75, 512] accumulated over 32 src-tiles
           (stationary bf16 h' tiles, moving fp8 count tiles, 512-wide
           streams; 5 PSUM banks), split in two src-m halves to chase
           the two AllGathers.
  Phase C: per (m,r): PE-transpose zT -> [dst,75], + root' + bias';
           per-row BN (bn_stats/bn_aggr) + gamma/beta + ReLU.
"""
import numpy as np
import ml_dtypes

import concourse.bacc as bacc
import concourse.mybir as mybir
import concourse.tile as tile
from concourse.bass_utils import run_bass_kernel_spmd
from concourse.masks import make_identity

P = 128
NCORES = 8
N = 4096          # nodes
U = 2048          # users
R = 5             # relations
H0 = 500
H1 = 75
EPS = 1e-5

NL = N // NCORES              # 512 node rows per core
KB_A = N // P                 # 32 contraction tiles, phase A
GB_A = 16                     # phase-A load groups
KPG = KB_A // GB_A            # 2 kb per group
WCOL = R * H1 + H1            # 450 folded-weight columns
MB = NL // P                  # 4 M-tiles per core
KT_B = R * MB                 # 20 k-tiles per core-block in phase B
WARM = 3                      # at4 chunks prefetched during phase A

F32 = mybir.dt.float32
BF16 = mybir.dt.bfloat16
FP8 = mybir.dt.float8e4
NP_FP8 = ml_dtypes.float8_e4m3

# test hooks
TRACE = False
LAST_RESULTS = None
_NC_CACHE = None


def _build():
    nc = bacc.Bacc("TRN2", target_bir_lowering=False, debug=False,
                   num_devices=NCORES)

    # host-swizzled inputs; layouts noted as [partition, free...]
    # x4[p, kb*NL + m] = x[coreRows m][i = kb*128+p]
    x4_d = nc.dram_tensor("x4", [P, KB_A * NL], BF16, kind="ExternalInput")
    # w4[p, kb*WCOL + j] = Wall'[kb*128+p, j]
    w4_d = nc.dram_tensor("w4", [P, KB_A * WCOL], BF16, kind="ExternalInput")
    # at4[p, (h, cb, r, mk2)*NL + d] = Mcnt[(r, src), myDst d];
    # src = cb*512 + (h*2+mk2)*128 + p  (fp8 counts, src-m-half major)
    at4_d = nc.dram_tensor("at4", [P, NCORES * KT_B * NL], FP8,
                           kind="ExternalInput")
    # cinvT[0, r*NL + d] = 1 / max(cnt[dst = d @ core, r], 1)
    cinvT_d = nc.dram_tensor("cinvT", [1, R * NL], BF16,
                             kind="ExternalInput")
    biasb_d = nc.dram_tensor("biasb", [P, H1], F32, kind="ExternalInput")
    gamma_d = nc.dram_tensor("gamma", [P, MB], F32, kind="ExternalInput")
    beta_d = nc.dram_tensor("beta", [P, MB], F32, kind="ExternalInput")
    out_d = nc.dram_tensor("out", [NL, H1], F32, kind="ExternalOutput")

    HHW = 2 * R * H1              # 750 h'-columns per AllGather half
    HB = R * 2 * NL               # at4 columns per (half, cb) chunk
    with tile.TileContext(nc) as tc:
        with (
            tc.tile_pool(name="big", bufs=1) as big,
            tc.tile_pool(name="xtp", bufs=GB_A) as xtp,
            tc.tile_pool(name="wtp", bufs=GB_A) as wtp,
            tc.tile_pool(name="io", bufs=4) as iop,
            tc.tile_pool(name="hhp", bufs=2 * NCORES) as hhp,
            tc.tile_pool(name="atp", bufs=2) as atp,
            tc.tile_pool(name="persist", bufs=4) as pp,
            tc.tile_pool(name="stp", bufs=5) as stp,
            tc.tile_pool(name="zp", bufs=2) as zp,
            tc.tile_pool(name="bn", bufs=4) as bnp,
            tc.tile_pool(name="psA", bufs=3, space="PSUM") as psa,
            tc.tile_pool(name="psB", bufs=5, space="PSUM") as psb,
            tc.tile_pool(name="dram", bufs=1, space="DRAM") as dramp,
        ):
            # small setup tiles first (cheap; keeps them off the tail)
            ones75 = big.tile([1, H1], BF16, tag="ones")
            nc.vector.memset(ones75, 1.0)
            eps_t = big.tile([P, 1], F32, tag="eps")
            nc.vector.memset(eps_t, EPS)
            ident = big.tile([P, P], F32, tag="ident")
            make_identity(nc, ident)
            cinvT = big.tile([1, R * NL], BF16, tag="cinvT")
            nc.scalar.dma_start(out=cinvT, in_=cinvT_d[:, :])
            biasb = big.tile([P, H1], F32, tag="bias")
            nc.scalar.dma_start(out=biasb, in_=biasb_d[:, :])
            gam = big.tile([P, MB], F32, tag="gam")
            nc.scalar.dma_start(out=gam, in_=gamma_d[:, :])
            bet = big.tile([P, MB], F32, tag="bet")
            nc.scalar.dma_start(out=bet, in_=beta_d[:, :])

            # PE warm-up: the HAM clock gate holds the array at 1.2GHz
            # until ~3.4us of sustained activity; burn that window on
            # dummy matmuls while the first x/W groups stream in, so
            # phase A runs at 2.4GHz from its first instruction
            wrm = big.tile([P, P], BF16, tag="wrm")
            nc.vector.memset(wrm, 0.0)
            psW = psa.tile([P, P], F32, tag="psA", name="psW")
            for i in range(36):
                nc.tensor.matmul(psW, wrm, wrm, start=True, stop=True)

            # ---------------- Phase A: h' = x_rows @ Wall' ----------------
            xg, wg = [], []
            for g in range(GB_A):
                xt = xtp.tile([P, KPG, NL], BF16, tag="xt", name=f"xt_{g}")
                nc.sync.dma_start(
                    out=xt, in_=x4_d[:, g * KPG * NL:(g + 1) * KPG * NL])
                xg.append(xt)
                wt = wtp.tile([P, KPG, WCOL], BF16, tag="wt", name=f"wt_{g}")
                nc.scalar.dma_start(
                    out=wt, in_=w4_d[:, g * KPG * WCOL:(g + 1) * KPG * WCOL])
                wg.append(wt)

            # at4 rides the software-DGE queue (exempt from the
            # collective's hardware-DMA drain barrier) in 16 chunks, each
            # WAW-gated (1-elem vector pre-write dep on hb16[0]) so
            # transfers start only once x/W are nearly done --
            # unthrottled at4 starves phase A's input streams.  Chunking
            # keeps the Q7 sequencer interruptible: a SWDGE dma_start
            # occupies it for the whole transfer, and one 10MB dma would
            # push the AllGather triggers out by ~30us.
            aa = [atp.tile([P, NCORES, R * 2, NL], FP8, tag="aa",
                           name=f"aa{h}") for h in range(2)]

            # h_cr[h][p, m2*375 + r*75 + j] = h'[(2h+m2)*128+p, r*75+j]
            h_cr = [dramp.tile([P, HHW], BF16, tag=f"h_c{h}",
                               name=f"h_cr{h}") for h in range(2)]
            h_ar = [dramp.tile([NCORES * P, HHW], BF16, tag=f"h_a{h}",
                               addr_space="Shared", name=f"h_ar{h}")
                    for h in range(2)]

            rootf, hb16 = [], []
            for mg in range(2):          # m-pairs: 2-bank ILP, 3 psA bufs
                ps_m = [psa.tile([P, WCOL], F32, tag="psA",
                                 name=f"psA_{mg}_{mi}") for mi in range(2)]
                for g in range(GB_A):
                    for kb in range(KPG):
                        for mi in range(2):
                            m = mg * 2 + mi
                            nc.tensor.matmul(
                                ps_m[mi],
                                xg[g][:, kb, m * P:(m + 1) * P],
                                wg[g][:, kb, :],
                                start=(g == 0 and kb == 0),
                                stop=(g == GB_A - 1 and kb == KPG - 1),
                            )
                for mi in range(2):
                    m = mg * 2 + mi
                    rf = pp.tile([P, H1], F32, tag="rootf", name=f"rootf_{m}")
                    nc.vector.tensor_copy(out=rf, in_=ps_m[mi][:, R * H1:])
                    rootf.append(rf)
                    hb = iop.tile([P, R * H1], BF16, tag="hout",
                                  name=f"hout_{m}")
                    nc.vector.tensor_copy(out=hb, in_=ps_m[mi][:, :R * H1])
                    hb16.append(hb)
                    # hardware-DGE store: drains in <1us, so the trigger
                    # below fires as soon as this m-pair's h' lands
                    nc.sync.dma_start(
                        out=h_cr[mg][:, mi * R * H1:(mi + 1) * R * H1],
                        in_=hb)
                # half-AllGather per m-pair: AG1's latency+wire hide
                # under phase A's second m-pair; AG2 queues on the CC
                # core behind AG1 and its wire hides under phase B's
                # first half.  high_priority slots each trigger between
                # at4 chunks on the Q7 the moment its h_cr dep fires.
                with tc.high_priority():
                    nc.gpsimd.collective_compute(
                        "AllGather",
                        mybir.AluOpType.bypass,
                        replica_groups=[list(range(NCORES))],
                        ins=[h_cr[mg][:, :]],
                        outs=[h_ar[mg][:, :]],
                    )
                if mg == 0:
                    # at4 in 16 chunks on the software-DGE queue, each
                    # WAW-gated (1-elem vector pre-write dep on hb16[0])
                    # so transfers start only once x/W are nearly done --
                    # unthrottled at4 starves phase A's input streams
                    for h in range(2):
                        for cb in range(NCORES):
                            nc.vector.tensor_copy(
                                out=aa[h][0:1, cb, 0, 0:1],
                                in_=hb16[0][0:1, 0:1])
                    for h in range(2):
                        for cb in range(NCORES):
                            base = (h * NCORES + cb) * HB
                            nc.gpsimd.dma_start(
                                out=aa[h][:, cb],
                                in_=at4_d[:, base:base + HB])

            # rank-1 broadcast rows B_r = ones[75] x cinv_r[512]; runs on
            # the PE between phase A and B (no phase-B dependency)
            brs = []
            for r in range(R):
                br = psa.tile([H1, NL], F32, tag="psA", name=f"br_{r}")
                nc.tensor.matmul(br, ones75, cinvT[:, r * NL:(r + 1) * NL],
                                 start=True, stop=True)
                bs = stp.tile([H1, NL], F32, tag="sT", name=f"brs_{r}")
                nc.vector.tensor_copy(out=bs, in_=br)
                brs.append(bs)

            # ------- Phase B: S_r = sum_s h'_r-tile.T @ Mcnt-tile ---------
            # half 1 (src m-tiles 0,1 of every core) follows AG1; hh rows
            # stream on the idle sync hardware queue at full rate
            hh = [[None] * NCORES for _ in range(2)]
            for cb in range(NCORES):
                t = hhp.tile([P, HHW], BF16, tag="hh", name=f"hh0_{cb}")
                nc.sync.dma_start(out=t, in_=h_ar[0][cb * P:(cb + 1) * P, :])
                hh[0][cb] = t
            psS = [psb.tile([H1, NL], F32, tag="psB", name=f"psB_{r}")
                   for r in range(R)]
            for cb in range(NCORES):
                for mk2 in range(2):
                    for r in range(R):
                        nc.tensor.matmul(
                            psS[r],
                            hh[0][cb][:, mk2 * R * H1 + r * H1:
                                      mk2 * R * H1 + (r + 1) * H1],
                            aa[0][:, cb, r * 2 + mk2, :],
                            start=(cb == 0 and mk2 == 0),
                            stop=False,
                        )

            # half 2 follows AG2, r-OUTER: each psS[r] retires after its
            # sweep so the zT = sum_r S_r o cinv_r combine overlaps the
            # remaining matmuls instead of serializing after phase B
            for cb in range(NCORES):
                t = hhp.tile([P, HHW], BF16, tag="hh", name=f"hh1_{cb}")
                nc.sync.dma_start(out=t, in_=h_ar[1][cb * P:(cb + 1) * P, :])
                hh[1][cb] = t
            zT = pp.tile([H1, NL], F32, tag="zT")
            # r-PAIRS: strict r-outer put 16 consecutive matmuls on one
            # PSUM bank (~320ns/mm vs 231 in the 5-bank half 1);
            # alternating two banks restores accumulation ILP while each
            # pair still retires early enough for the zT fold to overlap
            for rp in range((R + 1) // 2):
                rs = [r for r in (2 * rp, 2 * rp + 1) if r < R]
                for cb in range(NCORES):
                    for mk2 in range(2):
                        for r in rs:
                            nc.tensor.matmul(
                                psS[r],
                                hh[1][cb][:, mk2 * R * H1 + r * H1:
                                          mk2 * R * H1 + (r + 1) * H1],
                                aa[1][:, cb, r * 2 + mk2, :],
                                start=False,
                                stop=(cb == NCORES - 1 and mk2 == 1),
                            )
                # psS pair complete -> fold into zT while the next pair
                # accumulates
                for r in rs:
                    if r == 0:
                        nc.vector.tensor_mul(out=zT, in0=psS[0], in1=brs[0])
                    else:
                        ztmp = zp.tile([H1, NL], F32, tag="ztmp",
                                       name=f"ztmp_{r}")
                        nc.vector.tensor_mul(out=ztmp, in0=psS[r],
                                             in1=brs[r])
                        nc.vector.tensor_add(out=zT, in0=zT, in1=ztmp)

            # ---------------- Phase C: combine -> BN -> ReLU --------------
            for m in range(MB):
                pt = psa.tile([P, H1], F32, tag="psA", name=f"pt_{m}")
                nc.tensor.transpose(
                    pt, zT[:, m * P:(m + 1) * P], ident[:H1, :H1]
                )
                acc = bnp.tile([P, H1], F32, tag="acc", name=f"acc_{m}")
                nc.vector.tensor_add(out=acc, in0=pt, in1=rootf[m])
                nc.vector.tensor_add(out=acc, in0=acc, in1=biasb)

                stats = bnp.tile([P, 6], F32, tag="stats")
                nc.vector.bn_stats(out=stats, in_=acc)
                mv = bnp.tile([P, 2], F32, tag="mv")
                nc.vector.bn_aggr(out=mv, in_=stats)
                rstd = bnp.tile([P, 1], F32, tag="rstd")
                nc.scalar.activation(
                    out=rstd, in_=mv[:, 1:2],
                    func=mybir.ActivationFunctionType.Sqrt,
                    bias=eps_t, scale=1.0,
                )
                nc.vector.reciprocal(out=rstd, in_=rstd)
                g2 = bnp.tile([P, 1], F32, tag="g2")
                nc.vector.tensor_mul(out=g2, in0=rstd, in1=gam[:, m:m + 1])
                zt = bnp.tile([P, H1], F32, tag="zt")
                nc.vector.tensor_scalar(
                    out=zt, in0=acc,
                    scalar1=mv[:, 0:1], scalar2=g2,
                    op0=mybir.AluOpType.subtract, op1=mybir.AluOpType.mult,
                )
                nc.scalar.activation(
                    out=zt, in_=zt,
                    func=mybir.ActivationFunctionType.Relu,
                    bias=bet[:, m:m + 1], scale=1.0,
                )
                nc.scalar.dma_start(out=out_d[m * P:(m + 1) * P, :], in_=zt)

    nc.finalize()
    return nc


def _get_nc():
    global _NC_CACHE
    if _NC_CACHE is None:
        _NC_CACHE = _build()
    return _NC_CACHE


def kernel(**inputs) -> np.ndarray:
    global LAST_RESULTS
    x = np.asarray(inputs["x"], dtype=np.float32)
    basis = np.asarray(inputs["basis"], dtype=np.float32)
    comp = np.asarray(inputs["comp"], dtype=np.float32)
    root = np.asarray(inputs["root"], dtype=np.float32)
    bias_rgcn = np.asarray(inputs["bias_rgcn"], dtype=np.float32)
    fc_w = np.asarray(inputs["fc_w"], dtype=np.float32)
    bn_gamma_u = np.asarray(inputs["bn_gamma_u"], dtype=np.float32)
    bn_beta_u = np.asarray(inputs["bn_beta_u"], dtype=np.float32)
    bn_gamma_i = np.asarray(inputs["bn_gamma_i"], dtype=np.float32)
    bn_beta_i = np.asarray(inputs["bn_beta_i"], dtype=np.float32)
    edge_index = np.asarray(inputs["edge_index"]).astype(np.int64)
    edge_type = np.asarray(inputs["edge_type"]).astype(np.int64)

    src, dst = edge_index[0], edge_index[1]
    et = edge_type

    # Wall' = [W_r @ fc_w.T for r | root @ fc_w.T]  (fold the Dense layer)
    W = np.tensordot(comp, basis, axes=([1], [0]))          # [R, N, H0]
    Wp = np.einsum("rio,jo->rij", W, fc_w, optimize=True)   # [R, N, H1]
    wall = np.empty((N, WCOL), dtype=np.float32)
    wall[:, :R * H1] = Wp.transpose(1, 0, 2).reshape(N, R * H1)
    wall[:, R * H1:] = root @ fc_w.T
    w4 = np.ascontiguousarray(
        wall.astype(ml_dtypes.bfloat16)
        .reshape(KB_A, P, WCOL)                 # [kb, p, j]
        .transpose(1, 0, 2)                     # [p, kb, j]
        .reshape(P, KB_A * WCOL))

    xT16 = np.ascontiguousarray(x.T).astype(ml_dtypes.bfloat16)
    # x4[p, kb*NL + m] = x.T[kb*128+p, m@core]  (per-core slice below)
    x4_full = (xT16.reshape(KB_A, P, N)         # [kb, p, s]
               .transpose(1, 0, 2))             # [p, kb, s]

    # integer multiplicity matrix Mcnt[(r, src), dst] (exact in fp8e4m3)
    lin = (et * N + src) * np.int64(N) + dst
    cntmat = np.bincount(lin, minlength=R * N * N)
    assert cntmat.max() <= 16, "edge multiplicity too large for fp8 counts"
    atw = cntmat.astype(NP_FP8).reshape(R, NCORES, MB, P, N)

    # per-(dst, r) inverse counts
    cnt = np.bincount(dst * R + et, minlength=N * R).astype(np.float64)
    cinv_full = (1.0 / np.maximum(cnt, 1.0)).astype(np.float32).reshape(N, R)

    biasb = np.ascontiguousarray(
        np.broadcast_to(bias_rgcn @ fc_w.T, (P, H1)), dtype=np.float32)
    gamma_all = np.concatenate([bn_gamma_u, bn_gamma_i])
    beta_all = np.concatenate([bn_beta_u, bn_beta_i])

    in_maps = []
    for c in range(NCORES):
        sl = slice(c * NL, (c + 1) * NL)
        atc = atw[:, :, :, :, sl]               # [r, cb, mk, p, d]
        at4 = (atc.reshape(R, NCORES, 2, 2, P, NL)  # mk -> (h, mk2)
               .transpose(4, 2, 1, 0, 3, 5)         # [p, h, cb, r, mk2, d]
               .reshape(P, NCORES * KT_B * NL))
        in_maps.append({
            "x4": np.ascontiguousarray(
                x4_full[:, :, sl]).reshape(P, KB_A * NL),
            "w4": w4,
            "at4": np.ascontiguousarray(at4),
            "cinvT": np.ascontiguousarray(
                cinv_full[sl].T).astype(ml_dtypes.bfloat16).reshape(1, R * NL),
            "biasb": biasb,
            "gamma": np.ascontiguousarray(gamma_all[sl].reshape(MB, P).T),
            "beta": np.ascontiguousarray(beta_all[sl].reshape(MB, P).T),
        })

    nc = _get_nc()
    import os as _os
    _tc = (list(range(NCORES))
           if _os.environ.get("KTRACE_ALL") == "1" else None)
    res = run_bass_kernel_spmd(
        nc, in_maps, core_ids=list(range(NCORES)), trace=TRACE,
        trace_cores=_tc,
    )
    LAST_RESULTS = res

    z = np.concatenate([res.results[c]["out"] for c in range(NCORES)], axis=0)
    return np.stack([z[:U], z[U:]], axis=0)



# revision 29
# speedup vs baseline: 1.0870x; 1.0166x over previous
"""GCEncoder (RGCN basis-decomposition conv + mean aggregation + Dense/BN/ReLU)
as a Bass/Tile kernel on 8 Trainium2 NeuronCores.

Math (reference):
  W[r]  = sum_b comp[r,b] * basis[b]                    [R, N, H0]
  h[r]  = x @ W[r]                                      [R, N, H0]
  agg[d] = sum_r (1/cnt[d,r]) * sum_{e: dst=d, type=r} h[r, src_e]
  feats = agg + x @ root + bias
  z     = feats @ fc_w.T ; per-row batchnorm over H1 + gamma/beta + relu
  out   = (z[:U], z[U:]) stacked -> [2, U, H1]

Everything before the BN is linear in the H0 axis, so fc_w is folded into
the weights on the host: W'[r] = W[r] @ fc_w.T (4096 x 75), root' =
root @ fc_w.T, bias' = bias @ fc_w.T.  The device only moves 75-wide
features (~6.7x fewer matmul FLOPs than the unfolded form):

  z[d] = sum_r (1/cnt[d,r]) * (Mcnt_r[d,:] @ h'_r) + x[d] @ root' + bias'

with Mcnt_r the integer edge-multiplicity matrix (exact in fp8e4m3, half
the HBM bytes of a bf16 weighted adjacency) and h'_r = x @ W'_r.

Scheduling facts learned from perfetto traces on this stack:
  - the collective trigger waits for ALL in-flight hardware-DGE DMA to
    drain, so bulk at4 streaming rides the software-DGE (gpsimd) queue,
    which is exempt; the tiny h_cr stores ride the hardware queue so
    they drain fast and never queue behind at4 (the single-AG baseline
    lost ~35us to exactly that);
  - a collective costs ~11us trigger->mesh plus wire time; TWO half
    AllGathers pipeline: AG1 (m-tiles 0,1) triggers mid-phase-A and its
    wire time hides under the second m-pair's matmuls; AG2's hides
    under phase B's first half;
  - interleaved PSUM accumulation chains hide the per-matmul weight
    load (phase A: 2-bank pairs; phase B half 1: r-inner 5-bank ILP);
  - phase B half 2 runs r-OUTER so each psS[r] retires early and the
    zT = sum_r S_r o cinv_r combine overlaps the remaining matmuls,
    shrinking the serial BN tail.

Device strategy (per core c of 8, 512 node-rows each):
  Phase A: h'|root' = x[rows] @ [W'_0|..|W'_4|root'] (4096 x 450) in 16
           fine-grained load groups (x on sync-DGE, W' on scalar-DGE),
           m-tiles in 2 pair-passes (3 PSUM banks).  root' block stays
           in fp32 SBUF; h' -> bf16 SBUF -> h_cr[mg] DRAM (sync-DGE) ->
           192KB half-AllGather per m-pair.
  Phase B: per relation r: S_r[75, 512] accumulated over 32 src-tiles
           (stationary bf16 h' tiles, moving fp8 count tiles, 512-wide
           streams; 5 PSUM banks), split in two src-m halves to chase
           the two AllGathers.
  Phase C: per (m,r): PE-transpose zT -> [dst,75], + root' + bias';
           per-row BN (bn_stats/bn_aggr) + gamma/beta + ReLU.
"""
import numpy as np
import ml_dtypes

import concourse.bacc as bacc
import concourse.mybir as mybir
import concourse.tile as tile
from concourse.bass_utils import run_bass_kernel_spmd
from concourse.masks import make_identity

P = 128
NCORES = 8
N = 4096          # nodes
U = 2048          # users
R = 5             # relations
H0 = 500
H1 = 75
EPS = 1e-5

NL = N // NCORES              # 512 node rows per core
KB_A = N // P                 # 32 contraction tiles, phase A
GB_A = 16                     # phase-A load groups
KPG = KB_A // GB_A            # 2 kb per group
WCOL = R * H1 + H1            # 450 folded-weight columns
MB = NL // P                  # 4 M-tiles per core
KT_B = R * MB                 # 20 k-tiles per core-block in phase B
WARM = 3                      # at4 chunks prefetched during phase A

F32 = mybir.dt.float32
BF16 = mybir.dt.bfloat16
FP8 = mybir.dt.float8e4
NP_FP8 = ml_dtypes.float8_e4m3

# test hooks
TRACE = False
LAST_RESULTS = None
_NC_CACHE = None


def _build():
    nc = bacc.Bacc("TRN2", target_bir_lowering=False, debug=False,
                   num_devices=NCORES)

    # host-swizzled inputs; layouts noted as [partition, free...]
    # x4[p, kb*NL + m] = x[coreRows m][i = kb*128+p]
    x4_d = nc.dram_tensor("x4", [P, KB_A * NL], BF16, kind="ExternalInput")
    # w4[p, kb*WCOL + j] = Wall'[kb*128+p, j]
    w4_d = nc.dram_tensor("w4", [P, KB_A * WCOL], BF16, kind="ExternalInput")
    # at4[p, (h, cb, r, mk2)*NL + d] = Mcnt[(r, src), myDst d];
    # src = cb*512 + (h*2+mk2)*128 + p  (fp8 counts, src-m-half major)
    at4_d = nc.dram_tensor("at4", [P, NCORES * KT_B * NL], FP8,
                           kind="ExternalInput")
    # cinvT[0, r*NL + d] = 1 / max(cnt[dst = d @ core, r], 1)
    cinvT_d = nc.dram_tensor("cinvT", [1, R * NL], BF16,
                             kind="ExternalInput")
    biasb_d = nc.dram_tensor("biasb", [P, H1], F32, kind="ExternalInput")
    gamma_d = nc.dram_tensor("gamma", [P, MB], F32, kind="ExternalInput")
    beta_d = nc.dram_tensor("beta", [P, MB], F32, kind="ExternalInput")
    out_d = nc.dram_tensor("out", [NL, H1], F32, kind="ExternalOutput")

    HHW = 2 * R * H1              # 750 h'-columns per AllGather half
    HB = R * 2 * NL               # at4 columns per (half, cb) chunk
    with tile.TileContext(nc) as tc:
        with (
            tc.tile_pool(name="big", bufs=1) as big,
            tc.tile_pool(name="xtp", bufs=GB_A) as xtp,
            tc.tile_pool(name="wtp", bufs=GB_A) as wtp,
            tc.tile_pool(name="io", bufs=4) as iop,
            tc.tile_pool(name="hhp", bufs=2 * NCORES) as hhp,
            tc.tile_pool(name="atp", bufs=2) as atp,
            tc.tile_pool(name="persist", bufs=4) as pp,
            tc.tile_pool(name="stp", bufs=5) as stp,
            tc.tile_pool(name="zp", bufs=2) as zp,
            tc.tile_pool(name="bn", bufs=4) as bnp,
            tc.tile_pool(name="psA", bufs=3, space="PSUM") as psa,
            tc.tile_pool(name="psB", bufs=5, space="PSUM") as psb,
            tc.tile_pool(name="dram", bufs=1, space="DRAM") as dramp,
        ):
            # small setup tiles first (cheap; keeps them off the tail)
            ones75 = big.tile([1, H1], BF16, tag="ones")
            nc.vector.memset(ones75, 1.0)
            eps_t = big.tile([P, 1], F32, tag="eps")
            nc.vector.memset(eps_t, EPS)
            ident = big.tile([P, P], F32, tag="ident")
            make_identity(nc, ident)
            cinvT = big.tile([1, R * NL], BF16, tag="cinvT")
            nc.scalar.dma_start(out=cinvT, in_=cinvT_d[:, :])
            biasb = big.tile([P, H1], F32, tag="bias")
            nc.scalar.dma_start(out=biasb, in_=biasb_d[:, :])
            gam = big.tile([P, MB], F32, tag="gam")
            nc.scalar.dma_start(out=gam, in_=gamma_d[:, :])
            bet = big.tile([P, MB], F32, tag="bet")
            nc.scalar.dma_start(out=bet, in_=beta_d[:, :])

            # PE warm-up: the HAM clock gate holds the array at 1.2GHz
            # until ~3.4us of sustained activity; burn that window on
            # dummy matmuls while the first x/W groups stream in, so
            # phase A runs at 2.4GHz from its first instruction
            wrm = big.tile([P, P], BF16, tag="wrm")
            nc.vector.memset(wrm, 0.0)
            psW = psa.tile([P, P], F32, tag="psA", name="psW")
            for i in range(36):
                nc.tensor.matmul(psW, wrm, wrm, start=True, stop=True)

            # ---------------- Phase A: h' = x_rows @ Wall' ----------------
            xg, wg = [], []
            for g in range(GB_A):
                xt = xtp.tile([P, KPG, NL], BF16, tag="xt", name=f"xt_{g}")
                nc.sync.dma_start(
                    out=xt, in_=x4_d[:, g * KPG * NL:(g + 1) * KPG * NL])
                xg.append(xt)
                wt = wtp.tile([P, KPG, WCOL], BF16, tag="wt", name=f"wt_{g}")
                nc.scalar.dma_start(
                    out=wt, in_=w4_d[:, g * KPG * WCOL:(g + 1) * KPG * WCOL])
                wg.append(wt)

            # at4 rides the software-DGE queue (exempt from the
            # collective's hardware-DMA drain barrier) in 16 chunks, each
            # WAW-gated (1-elem vector pre-write dep on hb16[0]) so
            # transfers start only once x/W are nearly done --
            # unthrottled at4 starves phase A's input streams.  Chunking
            # keeps the Q7 sequencer interruptible: a SWDGE dma_start
            # occupies it for the whole transfer, and one 10MB dma would
            # push the AllGather triggers out by ~30us.
            aa = [atp.tile([P, NCORES, R * 2, NL], FP8, tag="aa",
                           name=f"aa{h}") for h in range(2)]

            # h_cr[h][p, m2*375 + r*75 + j] = h'[(2h+m2)*128+p, r*75+j]
            h_cr = [dramp.tile([P, HHW], BF16, tag=f"h_c{h}",
                               name=f"h_cr{h}") for h in range(2)]
            h_ar = [dramp.tile([NCORES * P, HHW], BF16, tag=f"h_a{h}",
                               addr_space="Shared", name=f"h_ar{h}")
                    for h in range(2)]

            rootf, hb16 = [], []
            for mg in range(2):          # m-pairs: 2-bank ILP, 3 psA bufs
                ps_m = [psa.tile([P, WCOL], F32, tag="psA",
                                 name=f"psA_{mg}_{mi}") for mi in range(2)]
                for g in range(GB_A):
                    for kb in range(KPG):
                        for mi in range(2):
                            m = mg * 2 + mi
                            nc.tensor.matmul(
                                ps_m[mi],
                                xg[g][:, kb, m * P:(m + 1) * P],
                                wg[g][:, kb, :],
                                start=(g == 0 and kb == 0),
                                stop=(g == GB_A - 1 and kb == KPG - 1),
                            )
                for mi in range(2):
                    m = mg * 2 + mi
                    rf = pp.tile([P, H1], F32, tag="rootf", name=f"rootf_{m}")
                    # fold bias' in here: drops one add from the BN tail
                    nc.vector.tensor_add(out=rf, in0=ps_m[mi][:, R * H1:],
                                         in1=biasb)
                    rootf.append(rf)
                    hb = iop.tile([P, R * H1], BF16, tag="hout",
                                  name=f"hout_{m}")
                    nc.vector.tensor_copy(out=hb, in_=ps_m[mi][:, :R * H1])
                    hb16.append(hb)
                    # hardware-DGE store: drains in <1us, so the trigger
                    # below fires as soon as this m-pair's h' lands
                    nc.sync.dma_start(
                        out=h_cr[mg][:, mi * R * H1:(mi + 1) * R * H1],
                        in_=hb)
                # half-AllGather per m-pair: AG1's latency+wire hide
                # under phase A's second m-pair; AG2 queues on the CC
                # core behind AG1 and its wire hides under phase B's
                # first half.  high_priority slots each trigger between
                # at4 chunks on the Q7 the moment its h_cr dep fires.
                with tc.high_priority():
                    nc.gpsimd.collective_compute(
                        "AllGather",
                        mybir.AluOpType.bypass,
                        replica_groups=[list(range(NCORES))],
                        ins=[h_cr[mg][:, :]],
                        outs=[h_ar[mg][:, :]],
                    )
                if mg == 0:
                    # at4 in 16 chunks on the software-DGE queue, each
                    # WAW-gated (1-elem vector pre-write dep on hb16[0])
                    # so transfers start only once x/W are nearly done --
                    # unthrottled at4 starves phase A's input streams
                    for h in range(2):
                        for cb in range(NCORES):
                            nc.vector.tensor_copy(
                                out=aa[h][0:1, cb, 0, 0:1],
                                in_=hb16[0][0:1, 0:1])
                    for h in range(2):
                        for cb in range(NCORES):
                            base = (h * NCORES + cb) * HB
                            nc.gpsimd.dma_start(
                                out=aa[h][:, cb],
                                in_=at4_d[:, base:base + HB])

            # rank-1 broadcast rows B_r = ones[75] x cinv_r[512]; runs on
            # the PE between phase A and B (no phase-B dependency)
            brs = []
            for r in range(R):
                br = psa.tile([H1, NL], F32, tag="psA", name=f"br_{r}")
                nc.tensor.matmul(br, ones75, cinvT[:, r * NL:(r + 1) * NL],
                                 start=True, stop=True)
                bs = stp.tile([H1, NL], F32, tag="sT", name=f"brs_{r}")
                nc.vector.tensor_copy(out=bs, in_=br)
                brs.append(bs)

            # ------- Phase B: S_r = sum_s h'_r-tile.T @ Mcnt-tile ---------
            # half 1 (src m-tiles 0,1 of every core) follows AG1; hh rows
            # stream on the idle sync hardware queue at full rate
            hh = [[None] * NCORES for _ in range(2)]
            for cb in range(NCORES):
                t = hhp.tile([P, HHW], BF16, tag="hh", name=f"hh0_{cb}")
                nc.sync.dma_start(out=t, in_=h_ar[0][cb * P:(cb + 1) * P, :])
                hh[0][cb] = t
            psS = [psb.tile([H1, NL], F32, tag="psB", name=f"psB_{r}")
                   for r in range(R)]
            for cb in range(NCORES):
                for mk2 in range(2):
                    for r in range(R):
                        nc.tensor.matmul(
                            psS[r],
                            hh[0][cb][:, mk2 * R * H1 + r * H1:
                                      mk2 * R * H1 + (r + 1) * H1],
                            aa[0][:, cb, r * 2 + mk2, :],
                            start=(cb == 0 and mk2 == 0),
                            stop=False,
                        )

            # half 2 follows AG2, r-OUTER: each psS[r] retires after its
            # sweep so the zT = sum_r S_r o cinv_r combine overlaps the
            # remaining matmuls instead of serializing after phase B
            for cb in range(NCORES):
                t = hhp.tile([P, HHW], BF16, tag="hh", name=f"hh1_{cb}")
                nc.sync.dma_start(out=t, in_=h_ar[1][cb * P:(cb + 1) * P, :])
                hh[1][cb] = t
            zT = pp.tile([H1, NL], F32, tag="zT")
            # r-PAIRS: strict r-outer put 16 consecutive matmuls on one
            # PSUM bank (~320ns/mm vs 231 in the 5-bank half 1);
            # alternating two banks restores accumulation ILP while each
            # pair still retires early enough for the zT fold to overlap
            for rs in [(0,), (1, 2), (3, 4)]:  # singleton sweep first:
                # the final sweeps keep two-bank accumulation ILP
                for cb in range(NCORES):
                    for mk2 in range(2):
                        for r in rs:
                            nc.tensor.matmul(
                                psS[r],
                                hh[1][cb][:, mk2 * R * H1 + r * H1:
                                          mk2 * R * H1 + (r + 1) * H1],
                                aa[1][:, cb, r * 2 + mk2, :],
                                start=False,
                                stop=(cb == NCORES - 1 and mk2 == 1),
                            )
                # psS pair complete -> fold into zT while the next pair
                # accumulates
                for r in rs:
                    if r == 0:
                        nc.vector.tensor_mul(out=zT, in0=psS[0], in1=brs[0])
                    else:
                        ztmp = zp.tile([H1, NL], F32, tag="ztmp",
                                       name=f"ztmp_{r}")
                        nc.vector.tensor_mul(out=ztmp, in0=psS[r],
                                             in1=brs[r])
                        nc.vector.tensor_add(out=zT, in0=zT, in1=ztmp)

            # ---------------- Phase C: combine -> BN -> ReLU --------------
            for m in range(MB):
                pt = psa.tile([P, H1], F32, tag="psA", name=f"pt_{m}")
                nc.tensor.transpose(
                    pt, zT[:, m * P:(m + 1) * P], ident[:H1, :H1]
                )
                acc = bnp.tile([P, H1], F32, tag="acc", name=f"acc_{m}")
                nc.vector.tensor_add(out=acc, in0=pt, in1=rootf[m])

                stats = bnp.tile([P, 6], F32, tag="stats")
                nc.vector.bn_stats(out=stats, in_=acc)
                mv = bnp.tile([P, 2], F32, tag="mv")
                nc.vector.bn_aggr(out=mv, in_=stats)
                rstd = bnp.tile([P, 1], F32, tag="rstd")
                nc.scalar.activation(
                    out=rstd, in_=mv[:, 1:2],
                    func=mybir.ActivationFunctionType.Sqrt,
                    bias=eps_t, scale=1.0,
                )
                nc.vector.reciprocal(out=rstd, in_=rstd)
                g2 = bnp.tile([P, 1], F32, tag="g2")
                nc.vector.tensor_mul(out=g2, in0=rstd, in1=gam[:, m:m + 1])
                zt = bnp.tile([P, H1], F32, tag="zt")
                nc.vector.tensor_scalar(
                    out=zt, in0=acc,
                    scalar1=mv[:, 0:1], scalar2=g2,
                    op0=mybir.AluOpType.subtract, op1=mybir.AluOpType.mult,
                )
                nc.scalar.activation(
                    out=zt, in_=zt,
                    func=mybir.ActivationFunctionType.Relu,
                    bias=bet[:, m:m + 1], scale=1.0,
                )
                nc.scalar.dma_start(out=out_d[m * P:(m + 1) * P, :], in_=zt)

    nc.finalize()
    return nc


def _get_nc():
    global _NC_CACHE
    if _NC_CACHE is None:
        _NC_CACHE = _build()
    return _NC_CACHE


def kernel(**inputs) -> np.ndarray:
    global LAST_RESULTS
    x = np.asarray(inputs["x"], dtype=np.float32)
    basis = np.asarray(inputs["basis"], dtype=np.float32)
    comp = np.asarray(inputs["comp"], dtype=np.float32)
    root = np.asarray(inputs["root"], dtype=np.float32)
    bias_rgcn = np.asarray(inputs["bias_rgcn"], dtype=np.float32)
    fc_w = np.asarray(inputs["fc_w"], dtype=np.float32)
    bn_gamma_u = np.asarray(inputs["bn_gamma_u"], dtype=np.float32)
    bn_beta_u = np.asarray(inputs["bn_beta_u"], dtype=np.float32)
    bn_gamma_i = np.asarray(inputs["bn_gamma_i"], dtype=np.float32)
    bn_beta_i = np.asarray(inputs["bn_beta_i"], dtype=np.float32)
    edge_index = np.asarray(inputs["edge_index"]).astype(np.int64)
    edge_type = np.asarray(inputs["edge_type"]).astype(np.int64)

    src, dst = edge_index[0], edge_index[1]
    et = edge_type

    # Wall' = [W_r @ fc_w.T for r | root @ fc_w.T]  (fold the Dense layer)
    W = np.tensordot(comp, basis, axes=([1], [0]))          # [R, N, H0]
    Wp = np.einsum("rio,jo->rij", W, fc_w, optimize=True)   # [R, N, H1]
    wall = np.empty((N, WCOL), dtype=np.float32)
    wall[:, :R * H1] = Wp.transpose(1, 0, 2).reshape(N, R * H1)
    wall[:, R * H1:] = root @ fc_w.T
    w4 = np.ascontiguousarray(
        wall.astype(ml_dtypes.bfloat16)
        .reshape(KB_A, P, WCOL)                 # [kb, p, j]
        .transpose(1, 0, 2)                     # [p, kb, j]
        .reshape(P, KB_A * WCOL))

    xT16 = np.ascontiguousarray(x.T).astype(ml_dtypes.bfloat16)
    # x4[p, kb*NL + m] = x.T[kb*128+p, m@core]  (per-core slice below)
    x4_full = (xT16.reshape(KB_A, P, N)         # [kb, p, s]
               .transpose(1, 0, 2))             # [p, kb, s]

    # integer multiplicity matrix Mcnt[(r, src), dst] (exact in fp8e4m3)
    lin = (et * N + src) * np.int64(N) + dst
    cntmat = np.bincount(lin, minlength=R * N * N)
    assert cntmat.max() <= 16, "edge multiplicity too large for fp8 counts"
    atw = cntmat.astype(NP_FP8).reshape(R, NCORES, MB, P, N)

    # per-(dst, r) inverse counts
    cnt = np.bincount(dst * R + et, minlength=N * R).astype(np.float64)
    cinv_full = (1.0 / np.maximum(cnt, 1.0)).astype(np.float32).reshape(N, R)

    biasb = np.ascontiguousarray(
        np.broadcast_to(bias_rgcn @ fc_w.T, (P, H1)), dtype=np.float32)
    gamma_all = np.concatenate([bn_gamma_u, bn_gamma_i])
    beta_all = np.concatenate([bn_beta_u, bn_beta_i])

    in_maps = []
    for c in range(NCORES):
        sl = slice(c * NL, (c + 1) * NL)
        atc = atw[:, :, :, :, sl]               # [r, cb, mk, p, d]
        at4 = (atc.reshape(R, NCORES, 2, 2, P, NL)  # mk -> (h, mk2)
               .transpose(4, 2, 1, 0, 3, 5)         # [p, h, cb, r, mk2, d]
               .reshape(P, NCORES * KT_B * NL))
        in_maps.append({
            "x4": np.ascontiguousarray(
                x4_full[:, :, sl]).reshape(P, KB_A * NL),
            "w4": w4,
            "at4": np.ascontiguousarray(at4),
            "cinvT": np.ascontiguousarray(
                cinv_full[sl].T).astype(ml_dtypes.bfloat16).reshape(1, R * NL),
            "biasb": biasb,
            "gamma": np.ascontiguousarray(gamma_all[sl].reshape(MB, P).T),
            "beta": np.ascontiguousarray(beta_all[sl].reshape(MB, P).T),
        })

    nc = _get_nc()
    import os as _os
    _tc = (list(range(NCORES))
           if _os.environ.get("KTRACE_ALL") == "1" else None)
    res = run_bass_kernel_spmd(
        nc, in_maps, core_ids=list(range(NCORES)), trace=TRACE,
        trace_cores=_tc,
    )
    LAST_RESULTS = res

    z = np.concatenate([res.results[c]["out"] for c in range(NCORES)], axis=0)
    return np.stack([z[:U], z[U:]], axis=0)

